# revision 13
# baseline (speedup 1.0000x reference)
"""DWT front-end as fused banded matmuls on the PE array.

Each output band is a linear map of x (reflect-pad + conv cascades + crops all
compose into one banded matrix per band). Per 128-sample position tile the map
is block-tridiagonal; interior blocks repeat every 128 positions, so only ~20
unique 128x128 fp16 blocks exist across all 4 bands. The kernel streams
x^T tiles (built on host) through the PE as lhsT and the weight blocks as rhs,
accumulating row-major [128 rows, 512 outs] chunks in PSUM, then evacuates to
fp16 SBUF and DMAs out.
"""
import numpy as np
from numpy.lib.stride_tricks import sliding_window_view

import concourse.bass as bass
import concourse.tile as tile
from concourse import bacc, mybir
from concourse.bass_utils import run_bass_kernel_spmd

F32 = mybir.dt.float32
F16 = mybir.dt.float16

LEVELS = 3
K = 8
DEC_LO = np.array([-0.0105974018, 0.0328830117, 0.0308413818, -0.1870348117,
                   -0.0279837694, 0.6308807679, 0.7148465706, 0.2303778133], np.float64)
DEC_HI = np.array([-0.2303778133, 0.7148465706, -0.6308807679, -0.0279837694,
                   0.1870348117, 0.0308413818, -0.0328830117, -0.0105974018], np.float64)
REC_LO = np.array([0.2303778133, 0.7148465706, 0.6308807679, -0.0279837694,
                   -0.1870348117, 0.0308413818, 0.0328830117, -0.0105974018], np.float64)
REC_HI = np.array([-0.0105974018, -0.0328830117, 0.0308413818, 0.1870348117,
                   -0.0279837694, -0.6308807679, 0.7148465706, -0.2303778133], np.float64)

L0 = 8192
N_CORES = 8
ROWS_PER_CORE = 256
T = 128            # position tile (matmul contraction)
NT = L0 // T       # 64
CW = 512           # psum chunk width (one bank)
NG = L0 // CW      # 16 chunks per band


# ---------------------------------------------------------------- host math
def _dwconv_s2(x, w):
    xp = np.pad(x, ((0, 0), (K - 1, K - 1)), mode="reflect")
    win = sliding_window_view(xp, K, axis=1)[:, ::2]
    return win @ w


def _dwconvT_s2(x, w):
    n, L = x.shape
    xd = np.zeros((n, 2 * L - 1 + 2 * (K - 1)), x.dtype)
    xd[:, K - 1:K - 1 + 2 * L - 1:2] = x
    win = sliding_window_view(xd, K, axis=1)
    return win @ w[::-1]


def _fit(out, target_len):
    L = out.shape[-1]
    if L > target_len:
        s = (L - target_len) // 2
        return out[:, s:s + target_len]
    if L < target_len:
        return np.pad(out, ((0, 0), (0, target_len - L)))
    return out


def _band_pipeline(x):
    approx = x
    details = []
    target_lens = []
    for _ in range(LEVELS):
        target_lens.append(approx.shape[-1])
        details.append(_dwconv_s2(approx, DEC_HI))
        approx = _dwconv_s2(approx, DEC_LO)

    def recon(band_idx):
        if band_idx == 0:
            rec = approx
            for lvl in reversed(range(LEVELS)):
                rec = _fit(_dwconvT_s2(rec, REC_LO), target_lens[lvl])
            return rec
        pick = LEVELS - band_idx
        rec = _fit(_dwconvT_s2(details[pick], REC_HI), target_lens[pick])
        for lvl in reversed(range(pick)):
            rec = _fit(_dwconvT_s2(rec, REC_LO), target_lens[lvl])
        return rec

    return np.stack([_fit(recon(i), L0) for i in range(LEVELS + 1)], axis=0)


def _build_R():
    Rs = [np.zeros((L0, L0), np.float32) for _ in range(4)]
    chunk = 2048
    for s in range(0, L0, chunk):
        I = np.zeros((chunk, L0), np.float32)
        I[np.arange(chunk), s + np.arange(chunk)] = 1.0
        out = _band_pipeline(I)
        for b in range(4):
            Rs[b][s:s + chunk] = out[b]
    return Rs


def _build_schedule():
    """Derive weight blocks + per-(t) matmul segments.

    Returns (wmat, segs_by_t, closes, first_touch) where
      wmat: [128, 128*nblk] fp16 weight matrix (canonical blocks)
      segs_by_t[t]: list of (b, g, pcol_lo, pcol_hi, wcol_lo, wcol_hi,
                             start, stop) matmul segments for position tile t
      closes[t]: list of (b, g) psum groups whose last matmul is at tile t
      first_touch[(b, g)]: first t touching the group
    """
    Rs = _build_R()
    blocks = []
    bmap = {}
    # per band, per 128-col chunk c: instances {t: (blk, span_lo, span_hi)}
    inst = [[dict() for _ in range(NT)] for _ in range(4)]
    for b in range(4):
        R = Rs[b]
        for c in range(NT):
            cols = R[:, c * T:(c + 1) * T]
            for t in range(NT):
                blk = cols[t * T:(t + 1) * T]
                if not np.any(blk):
                    continue
                h = np.float16(blk).tobytes()
                idx = bmap.get(h)
                if idx is None:
                    idx = len(blocks)
                    blocks.append(np.float16(blk))
                    bmap[h] = idx
                inst[b][c][t] = idx
    del Rs

    segs_by_t = [[] for _ in range(NT)]
    closes = [[] for _ in range(NT)]
    first_touch = {}
    for b in range(4):
        for c in range(NT):
            ts = sorted(inst[b][c])
            blk_of = inst[b][c]
            # per-column tmin/tmax within this 128-col chunk
            tmin = np.full(T, 10 ** 9, np.int64)
            tmax = np.full(T, -1, np.int64)
            for t in ts:
                blk = blocks[blk_of[t]]
                nz = np.any(blk != 0, axis=0)
                idxs = np.flatnonzero(nz)
                tmin[idxs] = np.minimum(tmin[idxs], t)
                tmax[idxs] = np.maximum(tmax[idxs], t)
            assert (tmax >= 0).all(), f"uncovered column in band {b} chunk {c}"
            g = c // (CW // T)
            key = (b, g)
            for t in ts:
                blk = blocks[blk_of[t]]
                nz = np.any(blk != 0, axis=0)
                # maximal runs of nonzero columns, split where first-touch
                # status changes (psum pending-zero is all-or-none per inst)
                j = 0
                while j < T:
                    if not nz[j]:
                        j += 1
                        continue
                    st = bool(tmin[j] == t)
                    j2 = j
                    while (j2 < T and nz[j2] and (tmin[j2] == t) == st):
                        j2 += 1
                    pcol = (c * T - g * CW) + j
                    wcol = blk_of[t] * T + j
                    segs_by_t[t].append((b, g, pcol, pcol + (j2 - j),
                                         wcol, wcol + (j2 - j), st, False))
                    j = j2
                if key not in first_touch:
                    first_touch[key] = t
            # group close: last t over the group's chunks
    last_t = {}
    for t in range(NT):
        for (b, g, *_rest) in segs_by_t[t]:
            last_t[(b, g)] = t
    for (b, g), t in last_t.items():
        closes[t].append((b, g))
    wmat = np.concatenate(blocks, axis=1)  # [128, nblk*128]
    return wmat, segs_by_t, closes, first_touch


_SCHED = None


def _get_sched():
    global _SCHED
    if _SCHED is None:
        _SCHED = _build_schedule()
    return _SCHED


# ---------------------------------------------------------------- bass build
def build_nc():
    wmat, segs_by_t, closes, first_touch = _get_sched()
    nblk_cols = wmat.shape[1]

    nc = bacc.Bacc("TRN2", target_bir_lowering=False, debug=False,
                   num_devices=N_CORES)
    xt_ap = nc.dram_tensor("xt", [L0, ROWS_PER_CORE], F16,
                           kind="ExternalInput").ap()
    w_ap = nc.dram_tensor("w", [T, nblk_cols], F16, kind="ExternalInput").ap()
    y_ap = nc.dram_tensor("y", [4, ROWS_PER_CORE, L0], F16,
                          kind="ExternalOutput").ap()

    with tile.TileContext(nc) as tc:
        with tc.tile_pool(name="bufs", bufs=1) as pool, \
             tc.tile_pool(name="ob", bufs=2) as obpool, \
             tc.tile_pool(name="ps", bufs=1, space="PSUM") as pspool:
            w_sb = pool.tile([T, nblk_cols], F16, tag="w")
            nc.scalar.dma_start(w_sb[:], w_ap[:, :])
            xt_sb = pool.tile([128, NT * ROWS_PER_CORE], F16, tag="xt")
            xt_src = xt_ap.rearrange("(t p) r -> p t r", p=128)
            HT = NT // 4
            for q in range(4):
                nc.sync.dma_start(
                    xt_sb[:, q * HT * ROWS_PER_CORE:(q + 1) * HT * ROWS_PER_CORE],
                    xt_src[:, q * HT:(q + 1) * HT, :])

            evac_engines = [nc.vector, nc.scalar]
            ev_i = 0
            dma_q = [nc.sync, nc.gpsimd]

            # psum start/stop are per 2KB zero region (the whole bank):
            # exactly one start (first matmul into the bank) and one stop
            # (last matmul) per (b, g) group.
            last_seg = {}
            for t in range(NT):
                for i, (b, g, *_r) in enumerate(segs_by_t[t]):
                    last_seg[(b, g)] = (t, i)

            for rt in range(2):
                y_sb = []
                for b in range(4):
                    yb = obpool.tile([128, L0], F16, tag=f"y{b}", name=f"y{b}")
                    y_sb.append(yb)
                ps_tiles = {}
                done_groups = [0] * 4
                for t in range(NT):
                    lhsT = xt_sb[:, t * ROWS_PER_CORE + rt * 128:
                                 t * ROWS_PER_CORE + rt * 128 + 128]
                    for i, (b, g, plo, phi, wlo, whi, _st, _sp) in enumerate(
                            segs_by_t[t]):
                        key = (b, g)
                        first = key not in ps_tiles
                        if first:
                            ps_tiles[key] = pspool.tile(
                                [128, CW], F32, tag=f"ps{b}_{g % 2}",
                                name=f"ps{b}_{g % 2}")
                        nc.tensor.matmul(ps_tiles[key][:, plo:phi], lhsT,
                                         w_sb[:, wlo:whi], start=first,
                                         stop=last_seg[key] == (t, i))
                    for (b, g) in closes[t]:
                        eng = evac_engines[ev_i % 2]
                        ev_i += 1
                        dst = y_sb[b][:, g * CW:(g + 1) * CW]
                        src = ps_tiles.pop((b, g))[:]
                        if eng is nc.scalar:
                            eng.copy(dst, src)
                        else:
                            eng.tensor_copy(dst, src)
                        done_groups[b] += 1
                        if done_groups[b] in (NG // 2, NG):
                            rows = slice(rt * 128, (rt + 1) * 128)
                            half = 0 if done_groups[b] == NG // 2 else 1
                            cols = slice(half * (L0 // 2), (half + 1) * (L0 // 2))
                            dma_q[(b + rt + half) % 2].dma_start(
                                y_ap[b, rows, cols], y_sb[b][:, cols])
    nc.compile()
    return nc


_NC = None


def _get_nc():
    global _NC
    if _NC is None:
        _NC = build_nc()
    return _NC


def shard_inputs(x):
    wmat = _get_sched()[0]
    rows = np.ascontiguousarray(x.reshape(-1, L0))
    out = []
    for c in range(N_CORES):
        shard = rows[c * ROWS_PER_CORE:(c + 1) * ROWS_PER_CORE]
        xt = np.ascontiguousarray(shard.astype(np.float16).T)
        out.append({"xt": xt, "w": wmat})
    return out


def unshard_outputs(results):
    out = np.empty((4, N_CORES * ROWS_PER_CORE, L0), np.float32)
    for c, r in enumerate(results):
        out[:, c * ROWS_PER_CORE:(c + 1) * ROWS_PER_CORE, :] = r["y"]
    return out.reshape(4, 16, 128, L0)


def kernel(x):
    x = np.asarray(x, np.float32)
    assert x.shape == (16, 128, L0), x.shape
    nc = _get_nc()
    res = run_bass_kernel_spmd(nc, shard_inputs(x), core_ids=list(range(N_CORES)))
    return unshard_outputs(res.results)


# revision 16
# speedup vs baseline: 1.4436x; 1.4436x over previous
"""DWT front-end as fused banded matmuls on the PE array.

Each output band is a linear map of x (reflect-pad + conv cascades + crops all
compose into one banded matrix per band). Per 128-sample position tile the map
is block-tridiagonal; interior blocks repeat every 128 positions, so only ~20
unique 128x128 fp16 blocks exist across all 4 bands. The kernel streams
x^T tiles (built on host) through the PE as lhsT and the weight blocks as rhs,
accumulating row-major [128 rows, 512 outs] chunks in PSUM, then evacuates to
fp16 SBUF and DMAs out.
"""
import numpy as np
from numpy.lib.stride_tricks import sliding_window_view

import concourse.bass as bass
import concourse.tile as tile
from concourse import bacc, mybir
from concourse.bass_utils import run_bass_kernel_spmd

F32 = mybir.dt.float32
F16 = mybir.dt.float16

LEVELS = 3
K = 8
DEC_LO = np.array([-0.0105974018, 0.0328830117, 0.0308413818, -0.1870348117,
                   -0.0279837694, 0.6308807679, 0.7148465706, 0.2303778133], np.float64)
DEC_HI = np.array([-0.2303778133, 0.7148465706, -0.6308807679, -0.0279837694,
                   0.1870348117, 0.0308413818, -0.0328830117, -0.0105974018], np.float64)
REC_LO = np.array([0.2303778133, 0.7148465706, 0.6308807679, -0.0279837694,
                   -0.1870348117, 0.0308413818, 0.0328830117, -0.0105974018], np.float64)
REC_HI = np.array([-0.0105974018, -0.0328830117, 0.0308413818, 0.1870348117,
                   -0.0279837694, -0.6308807679, 0.7148465706, -0.2303778133], np.float64)

L0 = 8192
N_CORES = 8
ROWS_PER_CORE = 256
T = 128            # position tile (matmul contraction)
NT = L0 // T       # 64
CW = 512           # psum chunk width (one bank)
NG = L0 // CW      # 16 chunks per band


# ---------------------------------------------------------------- host math
def _dwconv_s2(x, w):
    xp = np.pad(x, ((0, 0), (K - 1, K - 1)), mode="reflect")
    win = sliding_window_view(xp, K, axis=1)[:, ::2]
    return win @ w


def _dwconvT_s2(x, w):
    n, L = x.shape
    xd = np.zeros((n, 2 * L - 1 + 2 * (K - 1)), x.dtype)
    xd[:, K - 1:K - 1 + 2 * L - 1:2] = x
    win = sliding_window_view(xd, K, axis=1)
    return win @ w[::-1]


def _fit(out, target_len):
    L = out.shape[-1]
    if L > target_len:
        s = (L - target_len) // 2
        return out[:, s:s + target_len]
    if L < target_len:
        return np.pad(out, ((0, 0), (0, target_len - L)))
    return out


def _band_pipeline(x):
    approx = x
    details = []
    target_lens = []
    for _ in range(LEVELS):
        target_lens.append(approx.shape[-1])
        details.append(_dwconv_s2(approx, DEC_HI))
        approx = _dwconv_s2(approx, DEC_LO)

    def recon(band_idx):
        if band_idx == 0:
            rec = approx
            for lvl in reversed(range(LEVELS)):
                rec = _fit(_dwconvT_s2(rec, REC_LO), target_lens[lvl])
            return rec
        pick = LEVELS - band_idx
        rec = _fit(_dwconvT_s2(details[pick], REC_HI), target_lens[pick])
        for lvl in reversed(range(pick)):
            rec = _fit(_dwconvT_s2(rec, REC_LO), target_lens[lvl])
        return rec

    return np.stack([_fit(recon(i), L0) for i in range(LEVELS + 1)], axis=0)


def _build_R():
    Rs = [np.zeros((L0, L0), np.float32) for _ in range(4)]
    chunk = 2048
    for s in range(0, L0, chunk):
        I = np.zeros((chunk, L0), np.float32)
        I[np.arange(chunk), s + np.arange(chunk)] = 1.0
        out = _band_pipeline(I)
        for b in range(4):
            Rs[b][s:s + chunk] = out[b]
    return Rs


def _build_schedule():
    """Derive weight blocks + per-(t) matmul segments.

    Returns (wmat, segs_by_t, closes, first_touch) where
      wmat: [128, 128*nblk] fp16 weight matrix (canonical blocks)
      segs_by_t[t]: list of (b, g, pcol_lo, pcol_hi, wcol_lo, wcol_hi,
                             start, stop) matmul segments for position tile t
      closes[t]: list of (b, g) psum groups whose last matmul is at tile t
      first_touch[(b, g)]: first t touching the group
    """
    Rs = _build_R()
    blocks = []
    bmap = {}
    # per band, per 128-col chunk c: instances {t: (blk, span_lo, span_hi)}
    inst = [[dict() for _ in range(NT)] for _ in range(4)]
    for b in range(4):
        R = Rs[b]
        for c in range(NT):
            cols = R[:, c * T:(c + 1) * T]
            for t in range(NT):
                blk = cols[t * T:(t + 1) * T]
                if not np.any(blk):
                    continue
                h = np.float16(blk).tobytes()
                idx = bmap.get(h)
                if idx is None:
                    idx = len(blocks)
                    blocks.append(np.float16(blk))
                    bmap[h] = idx
                inst[b][c][t] = idx
    del Rs

    segs_by_t = [[] for _ in range(NT)]
    closes = [[] for _ in range(NT)]
    first_touch = {}
    for b in range(4):
        for c in range(NT):
            ts = sorted(inst[b][c])
            blk_of = inst[b][c]
            # per-column tmin/tmax within this 128-col chunk
            tmin = np.full(T, 10 ** 9, np.int64)
            tmax = np.full(T, -1, np.int64)
            for t in ts:
                blk = blocks[blk_of[t]]
                nz = np.any(blk != 0, axis=0)
                idxs = np.flatnonzero(nz)
                tmin[idxs] = np.minimum(tmin[idxs], t)
                tmax[idxs] = np.maximum(tmax[idxs], t)
            assert (tmax >= 0).all(), f"uncovered column in band {b} chunk {c}"
            g = c // (CW // T)
            key = (b, g)
            for t in ts:
                blk = blocks[blk_of[t]]
                nz = np.any(blk != 0, axis=0)
                # maximal runs of nonzero columns, split where first-touch
                # status changes (psum pending-zero is all-or-none per inst)
                j = 0
                while j < T:
                    if not nz[j]:
                        j += 1
                        continue
                    st = bool(tmin[j] == t)
                    j2 = j
                    while (j2 < T and nz[j2] and (tmin[j2] == t) == st):
                        j2 += 1
                    pcol = (c * T - g * CW) + j
                    wcol = blk_of[t] * T + j
                    segs_by_t[t].append((b, g, pcol, pcol + (j2 - j),
                                         wcol, wcol + (j2 - j), st, False))
                    j = j2
                if key not in first_touch:
                    first_touch[key] = t
            # group close: last t over the group's chunks
    last_t = {}
    for t in range(NT):
        for (b, g, *_rest) in segs_by_t[t]:
            last_t[(b, g)] = t
    for (b, g), t in last_t.items():
        closes[t].append((b, g))
    wmat = np.concatenate(blocks, axis=1)  # [128, nblk*128]
    return wmat, segs_by_t, closes, first_touch


_SCHED = None


def _get_sched():
    global _SCHED
    if _SCHED is None:
        _SCHED = _build_schedule()
    return _SCHED


# ---------------------------------------------------------------- bass build
def build_nc():
    wmat, segs_by_t, closes, first_touch = _get_sched()
    nblk_cols = wmat.shape[1]

    nc = bacc.Bacc("TRN2", target_bir_lowering=False, debug=False,
                   num_devices=N_CORES)
    xt_ap = nc.dram_tensor("xt", [128, NT * ROWS_PER_CORE], F16,
                           kind="ExternalInput").ap()
    w_ap = nc.dram_tensor("w", [T, nblk_cols], F16, kind="ExternalInput").ap()
    y_ap = nc.dram_tensor("y", [4, ROWS_PER_CORE, L0], F16,
                          kind="ExternalOutput").ap()

    with tile.TileContext(nc) as tc:
        with tc.tile_pool(name="bufs", bufs=1) as pool, \
             tc.tile_pool(name="ob", bufs=2) as obpool, \
             tc.tile_pool(name="ps", bufs=1, space="PSUM") as pspool:
            w_sb = pool.tile([T, nblk_cols], F16, tag="w")
            nc.scalar.dma_start(w_sb[:], w_ap[:, :])
            xt_sb = pool.tile([128, NT * ROWS_PER_CORE], F16, tag="xt")
            QW = NT * ROWS_PER_CORE // 4
            for q in range(4):
                nc.sync.dma_start(xt_sb[:, q * QW:(q + 1) * QW],
                                  xt_ap[:, q * QW:(q + 1) * QW])

            evac_engines = [nc.vector, nc.scalar]
            ev_i = 0
            dma_q = [nc.sync, nc.gpsimd]

            # psum start/stop are per 2KB zero region (the whole bank):
            # exactly one start (first matmul into the bank) and one stop
            # (last matmul) per (b, g) group.
            last_seg = {}
            for t in range(NT):
                for i, (b, g, *_r) in enumerate(segs_by_t[t]):
                    last_seg[(b, g)] = (t, i)

            for rt in range(2):
                y_sb = []
                for b in range(4):
                    yb = obpool.tile([128, L0], F16, tag=f"y{b}", name=f"y{b}")
                    y_sb.append(yb)
                ps_tiles = {}
                done_groups = [0] * 4
                for t in range(NT):
                    lhsT = xt_sb[:, t * ROWS_PER_CORE + rt * 128:
                                 t * ROWS_PER_CORE + rt * 128 + 128]
                    for i, (b, g, plo, phi, wlo, whi, _st, _sp) in enumerate(
                            segs_by_t[t]):
                        key = (b, g)
                        first = key not in ps_tiles
                        if first:
                            ps_tiles[key] = pspool.tile(
                                [128, CW], F32, tag=f"ps{b}_{g % 2}",
                                name=f"ps{b}_{g % 2}")
                        nc.tensor.matmul(ps_tiles[key][:, plo:phi], lhsT,
                                         w_sb[:, wlo:whi], start=first,
                                         stop=last_seg[key] == (t, i))
                    for (b, g) in closes[t]:
                        eng = evac_engines[ev_i % 2]
                        ev_i += 1
                        dst = y_sb[b][:, g * CW:(g + 1) * CW]
                        src = ps_tiles.pop((b, g))[:]
                        if eng is nc.scalar:
                            eng.copy(dst, src)
                        else:
                            eng.tensor_copy(dst, src)
                        done_groups[b] += 1
                        if done_groups[b] in (NG // 2, NG):
                            rows = slice(rt * 128, (rt + 1) * 128)
                            half = 0 if done_groups[b] == NG // 2 else 1
                            cols = slice(half * (L0 // 2), (half + 1) * (L0 // 2))
                            dma_q[(b + rt + half) % 2].dma_start(
                                y_ap[b, rows, cols], y_sb[b][:, cols])
    nc.compile()
    return nc


_NC = None


def _get_nc():
    global _NC
    if _NC is None:
        _NC = build_nc()
    return _NC


def shard_inputs(x):
    wmat = _get_sched()[0]
    rows = np.ascontiguousarray(x.reshape(-1, L0))
    out = []
    for c in range(N_CORES):
        shard = rows[c * ROWS_PER_CORE:(c + 1) * ROWS_PER_CORE]
        # pre-shuffle to the sbuf layout [p, t*256 + r] = x[r, 128t + p]
        xt = np.ascontiguousarray(
            shard.astype(np.float16).reshape(ROWS_PER_CORE, NT, T)
            .transpose(2, 1, 0).reshape(T, NT * ROWS_PER_CORE))
        out.append({"xt": xt, "w": wmat})
    return out


def unshard_outputs(results):
    out = np.empty((4, N_CORES * ROWS_PER_CORE, L0), np.float32)
    for c, r in enumerate(results):
        out[:, c * ROWS_PER_CORE:(c + 1) * ROWS_PER_CORE, :] = r["y"]
    return out.reshape(4, 16, 128, L0)


def kernel(x):
    x = np.asarray(x, np.float32)
    assert x.shape == (16, 128, L0), x.shape
    nc = _get_nc()
    res = run_bass_kernel_spmd(nc, shard_inputs(x), core_ids=list(range(N_CORES)))
    return unshard_outputs(res.results)


# revision 22
# speedup vs baseline: 2.5976x; 1.7994x over previous
"""DWT front-end as fused banded matmuls on the PE array.

Each output band is a linear map of x (reflect-pad + conv cascades + crops all
compose into one banded matrix per band). Per 128-sample position tile the map
is block-tridiagonal; interior blocks repeat every 128 positions, so only ~20
unique 128x128 fp16 blocks exist across all 4 bands. The kernel streams
x^T tiles (built on host) through the PE as lhsT and the weight blocks as rhs,
accumulating row-major [128 rows, 512 outs] chunks in PSUM, then evacuates to
fp16 SBUF and DMAs out.
"""
import numpy as np
from numpy.lib.stride_tricks import sliding_window_view

import concourse.bass as bass
import concourse.tile as tile
from concourse import bacc, mybir
from concourse.bass_utils import run_bass_kernel_spmd

F32 = mybir.dt.float32
F16 = mybir.dt.float16

LEVELS = 3
K = 8
DEC_LO = np.array([-0.0105974018, 0.0328830117, 0.0308413818, -0.1870348117,
                   -0.0279837694, 0.6308807679, 0.7148465706, 0.2303778133], np.float64)
DEC_HI = np.array([-0.2303778133, 0.7148465706, -0.6308807679, -0.0279837694,
                   0.1870348117, 0.0308413818, -0.0328830117, -0.0105974018], np.float64)
REC_LO = np.array([0.2303778133, 0.7148465706, 0.6308807679, -0.0279837694,
                   -0.1870348117, 0.0308413818, 0.0328830117, -0.0105974018], np.float64)
REC_HI = np.array([-0.0105974018, -0.0328830117, 0.0308413818, 0.1870348117,
                   -0.0279837694, -0.6308807679, 0.7148465706, -0.2303778133], np.float64)

L0 = 8192
N_CORES = 8
ROWS_PER_CORE = 256
T = 128            # position tile (matmul contraction)
NT = L0 // T       # 64
CW = 512           # psum chunk width (one bank)
NG = L0 // CW      # 16 chunks per band


# ---------------------------------------------------------------- host math
def _dwconv_s2(x, w):
    xp = np.pad(x, ((0, 0), (K - 1, K - 1)), mode="reflect")
    win = sliding_window_view(xp, K, axis=1)[:, ::2]
    return win @ w


def _dwconvT_s2(x, w):
    n, L = x.shape
    xd = np.zeros((n, 2 * L - 1 + 2 * (K - 1)), x.dtype)
    xd[:, K - 1:K - 1 + 2 * L - 1:2] = x
    win = sliding_window_view(xd, K, axis=1)
    return win @ w[::-1]


def _fit(out, target_len):
    L = out.shape[-1]
    if L > target_len:
        s = (L - target_len) // 2
        return out[:, s:s + target_len]
    if L < target_len:
        return np.pad(out, ((0, 0), (0, target_len - L)))
    return out


def _band_pipeline(x):
    approx = x
    details = []
    target_lens = []
    for _ in range(LEVELS):
        target_lens.append(approx.shape[-1])
        details.append(_dwconv_s2(approx, DEC_HI))
        approx = _dwconv_s2(approx, DEC_LO)

    def recon(band_idx):
        if band_idx == 0:
            rec = approx
            for lvl in reversed(range(LEVELS)):
                rec = _fit(_dwconvT_s2(rec, REC_LO), target_lens[lvl])
            return rec
        pick = LEVELS - band_idx
        rec = _fit(_dwconvT_s2(details[pick], REC_HI), target_lens[pick])
        for lvl in reversed(range(pick)):
            rec = _fit(_dwconvT_s2(rec, REC_LO), target_lens[lvl])
        return rec

    return np.stack([_fit(recon(i), L0) for i in range(LEVELS + 1)], axis=0)


def _build_R():
    Rs = [np.zeros((L0, L0), np.float32) for _ in range(4)]
    chunk = 2048
    for s in range(0, L0, chunk):
        I = np.zeros((chunk, L0), np.float32)
        I[np.arange(chunk), s + np.arange(chunk)] = 1.0
        out = _band_pipeline(I)
        for b in range(4):
            Rs[b][s:s + chunk] = out[b]
    return Rs


def _build_schedule():
    """Derive weight blocks + per-(t) matmul segments.

    Returns (wmat, segs_by_t, closes, first_touch) where
      wmat: [128, 128*nblk] fp16 weight matrix (canonical blocks)
      segs_by_t[t]: list of (b, g, pcol_lo, pcol_hi, wcol_lo, wcol_hi,
                             start, stop) matmul segments for position tile t
      closes[t]: list of (b, g) psum groups whose last matmul is at tile t
      first_touch[(b, g)]: first t touching the group
    """
    Rs = _build_R()
    blocks = []
    bmap = {}
    # per band, per 128-col chunk c: instances {t: (blk, span_lo, span_hi)}
    inst = [[dict() for _ in range(NT)] for _ in range(4)]
    for b in range(4):
        R = Rs[b]
        for c in range(NT):
            cols = R[:, c * T:(c + 1) * T]
            for t in range(NT):
                blk = cols[t * T:(t + 1) * T]
                if not np.any(blk):
                    continue
                h = np.float16(blk).tobytes()
                idx = bmap.get(h)
                if idx is None:
                    idx = len(blocks)
                    blocks.append(np.float16(blk))
                    bmap[h] = idx
                inst[b][c][t] = idx
    del Rs

    segs_by_t = [[] for _ in range(NT)]
    closes = [[] for _ in range(NT)]
    first_touch = {}
    for b in range(4):
        for c in range(NT):
            ts = sorted(inst[b][c])
            blk_of = inst[b][c]
            # per-column tmin/tmax within this 128-col chunk
            tmin = np.full(T, 10 ** 9, np.int64)
            tmax = np.full(T, -1, np.int64)
            for t in ts:
                blk = blocks[blk_of[t]]
                nz = np.any(blk != 0, axis=0)
                idxs = np.flatnonzero(nz)
                tmin[idxs] = np.minimum(tmin[idxs], t)
                tmax[idxs] = np.maximum(tmax[idxs], t)
            assert (tmax >= 0).all(), f"uncovered column in band {b} chunk {c}"
            g = c // (CW // T)
            key = (b, g)
            for t in ts:
                blk = blocks[blk_of[t]]
                nz = np.any(blk != 0, axis=0)
                # maximal runs of nonzero columns, split where first-touch
                # status changes (psum pending-zero is all-or-none per inst)
                j = 0
                while j < T:
                    if not nz[j]:
                        j += 1
                        continue
                    st = bool(tmin[j] == t)
                    j2 = j
                    while (j2 < T and nz[j2] and (tmin[j2] == t) == st):
                        j2 += 1
                    pcol = (c * T - g * CW) + j
                    wcol = blk_of[t] * T + j
                    segs_by_t[t].append((b, g, pcol, pcol + (j2 - j),
                                         wcol, wcol + (j2 - j), st, False))
                    j = j2
                if key not in first_touch:
                    first_touch[key] = t
            # group close: last t over the group's chunks
    last_t = {}
    for t in range(NT):
        for (b, g, *_rest) in segs_by_t[t]:
            last_t[(b, g)] = t
    for (b, g), t in last_t.items():
        closes[t].append((b, g))
    wmat = np.concatenate(blocks, axis=1)  # [128, nblk*128]
    return wmat, segs_by_t, closes, first_touch


_SCHED = None


def _get_sched():
    global _SCHED
    if _SCHED is None:
        _SCHED = _build_schedule()
    return _SCHED


# ---------------------------------------------------------------- bass build
def build_nc():
    wmat, segs_by_t, closes, first_touch = _get_sched()
    nblk_cols = wmat.shape[1]

    nc = bacc.Bacc("TRN2", target_bir_lowering=False, debug=False,
                   num_devices=N_CORES)
    xt_ap = nc.dram_tensor("xt", [128, NT * ROWS_PER_CORE], F16,
                           kind="ExternalInput").ap()
    w_ap = nc.dram_tensor("w", [T, nblk_cols], F16, kind="ExternalInput").ap()
    y_ap = nc.dram_tensor("y", [4, ROWS_PER_CORE, L0], F16,
                          kind="ExternalOutput").ap()

    with tile.TileContext(nc) as tc:
        with tc.tile_pool(name="bufs", bufs=1) as pool, \
             tc.tile_pool(name="ob", bufs=2) as obpool, \
             tc.tile_pool(name="ps", bufs=1, space="PSUM") as pspool:
            w_sb = pool.tile([T, nblk_cols], F16, tag="w")
            nc.scalar.dma_start(w_sb[:], w_ap[:, :])
            xt_sb = pool.tile([128, NT * ROWS_PER_CORE], F16, tag="xt")
            QW = NT * ROWS_PER_CORE // 8
            for q in range(8):
                nc.sync.dma_start(xt_sb[:, q * QW:(q + 1) * QW],
                                  xt_ap[:, q * QW:(q + 1) * QW])

            evac_engines = [nc.vector, nc.scalar]
            ev_i = 0
            dma_q = [nc.sync, nc.gpsimd]

            # psum start/stop are per 2KB zero region (the whole bank):
            # exactly one start (first matmul into the bank) and one stop
            # (last matmul) per (b, g) group.
            last_seg = {}
            for t in range(NT):
                for i, (b, g, *_r) in enumerate(segs_by_t[t]):
                    last_seg[(b, g)] = (t, i)

            for rt in range(2):
                y_sb = []
                for b in range(4):
                    yb = obpool.tile([128, L0], F16, tag=f"y{b}", name=f"y{b}")
                    y_sb.append(yb)
                ps_tiles = {}
                done_groups = [0] * 4
                for t in range(NT):
                    lhsT = xt_sb[:, t * ROWS_PER_CORE + rt * 128:
                                 t * ROWS_PER_CORE + rt * 128 + 128]
                    for i, (b, g, plo, phi, wlo, whi, _st, _sp) in enumerate(
                            segs_by_t[t]):
                        key = (b, g)
                        first = key not in ps_tiles
                        if first:
                            ps_tiles[key] = pspool.tile(
                                [128, CW], F32, tag=f"ps{b}_{g % 2}",
                                name=f"ps{b}_{g % 2}")
                        nc.tensor.matmul(ps_tiles[key][:, plo:phi], lhsT,
                                         w_sb[:, wlo:whi], start=first,
                                         stop=last_seg[key] == (t, i))
                    for (b, g) in closes[t]:
                        eng = evac_engines[ev_i % 2]
                        ev_i += 1
                        dst = y_sb[b][:, g * CW:(g + 1) * CW]
                        src = ps_tiles.pop((b, g))[:]
                        if eng is nc.scalar:
                            eng.copy(dst, src)
                        else:
                            eng.tensor_copy(dst, src)
                        done_groups[b] += 1
                        # taper the drain: big DMA once half done, then
                        # smaller pieces so the post-last-matmul tail is short
                        taper = {NG // 2: (0, NG // 2), 12: (NG // 2, 12),
                                 14: (12, 14), NG: (14, NG)}
                        if done_groups[b] in taper:
                            g0, g1 = taper[done_groups[b]]
                            rows = slice(rt * 128, (rt + 1) * 128)
                            cols = slice(g0 * CW, g1 * CW)
                            dma_q[(b + rt + g0) % 2].dma_start(
                                y_ap[b, rows, cols], y_sb[b][:, cols])
    nc.compile()
    return nc


_NC = None


def _get_nc():
    global _NC
    if _NC is None:
        _NC = build_nc()
    return _NC


def shard_inputs(x):
    wmat = _get_sched()[0]
    rows = np.ascontiguousarray(x.reshape(-1, L0))
    out = []
    for c in range(N_CORES):
        shard = rows[c * ROWS_PER_CORE:(c + 1) * ROWS_PER_CORE]
        # pre-shuffle to the sbuf layout [p, t*256 + r] = x[r, 128t + p]
        xt = np.ascontiguousarray(
            shard.astype(np.float16).reshape(ROWS_PER_CORE, NT, T)
            .transpose(2, 1, 0).reshape(T, NT * ROWS_PER_CORE))
        out.append({"xt": xt, "w": wmat})
    return out


def unshard_outputs(results):
    out = np.empty((4, N_CORES * ROWS_PER_CORE, L0), np.float32)
    for c, r in enumerate(results):
        out[:, c * ROWS_PER_CORE:(c + 1) * ROWS_PER_CORE, :] = r["y"]
    return out.reshape(4, 16, 128, L0)


def kernel(x):
    x = np.asarray(x, np.float32)
    assert x.shape == (16, 128, L0), x.shape
    nc = _get_nc()
    res = run_bass_kernel_spmd(nc, shard_inputs(x), core_ids=list(range(N_CORES)))
    return unshard_outputs(res.results)
